# revision 1
# baseline (speedup 1.0000x reference)
"""Spatial multi-head attention kernel for Trainium2 (8 NeuronCores).

Problem: B=8, T=64, N=170 nodes, C=128 channels, H=8 heads, D=16.
Attention over nodes N, independent per (b, t, h).

Strategy:
- Pure data-parallel over B: core b computes batch b fully (no collectives).
- Host transposes inputs to channel-major [B, T, C, N]; output returned
  channel-major [T, C, N] per core and transposed back on host.
- Per (b, t), everything stays channel-major on the device:
  * q/k projections emitted in a "padded head" layout: heads at 32-aligned
    partition bases (4 heads per [128, N] tile) so the K=16 energy matmuls
    are legal row-tiled matmuls running 4 heads concurrently.
  * softmax without max-subtraction (|energy/sqrt(C)| < ~3 in f32): exp on
    ACT directly PSUM -> SBUF, denominators via a ones-column in v_aug.
  * attn@v as outT[d, q] via col-tiled matmuls (M=17).
  * denominator reciprocal as exp(-log(x)) on ACT (DVE reciprocal is
    8 cyc/elem), expanded to [128, 2N] with a constant selector matmul.
  * final projection with head-scattered Wo''; output bias added during the
    PSUM evacuation on ACT.
- Matmul operands in bf16 (PSUM accumulation stays fp32): fp32 matmuls on
  TRN2 run as two HW passes, bf16 runs one and enables fast weight load.
  Set USE_BF16 = False for a full-fp32 build (~2.5x slower, rel err 4e-6
  instead of ~2e-4).
"""
import sys

sys.path.insert(0, "/opt/trn_rl_repo")

import numpy as np

import concourse.bacc as bacc
import concourse.mybir as mybir
import concourse.tile as tile
from concourse.bass_utils import run_bass_kernel_spmd

B, T, N, C = 8, 64, 170, 128
H, D = 8, 16
F32 = np.float32
DT = mybir.dt.float32
EXP = mybir.ActivationFunctionType.Exp
LOG = mybir.ActivationFunctionType.Ln
IDENT = mybir.ActivationFunctionType.Identity

USE_BF16 = True
MDT = mybir.dt.bfloat16 if USE_BF16 else mybir.dt.float32

NC0 = 128          # first k-chunk size
NC1 = N - NC0      # 42
PW = 340           # per-head psum span: chunk0 [0:170], chunk1 [170:340]
BANK = 512         # psum bank free size (f32)
TB = 2             # timesteps per DMA batch

_cache = {}


def _build_kernel():
    nc = bacc.Bacc("TRN2", target_bir_lowering=False, debug=False)

    q_in = nc.dram_tensor("q_in", [T, C, N], MDT, kind="ExternalInput")
    k_in = nc.dram_tensor("k_in", [T, C, N], MDT, kind="ExternalInput")
    v_in = nc.dram_tensor("v_in", [T, C, N], MDT, kind="ExternalInput")
    w_names = ["wqa", "wqb", "wka", "wkb", "wvt", "woa", "wob"]
    w_dram = {n: nc.dram_tensor(n, [C, C], MDT, kind="ExternalInput") for n in w_names}
    sel_dram = nc.dram_tensor("sel", [4, C], DT, kind="ExternalInput")
    cb_dram = nc.dram_tensor("cb", [C, 1], DT, kind="ExternalInput")
    out_dram = nc.dram_tensor("out", [T, C, N], DT, kind="ExternalOutput")

    with tile.TileContext(nc) as tc:
        with (
            tc.tile_pool(name="wp", bufs=1) as wp,
            tc.tile_pool(name="io", bufs=3) as io,
            tc.tile_pool(name="work", bufs=2) as work,
            tc.tile_pool(name="pt", bufs=2) as ptp,
            tc.tile_pool(name="eps", bufs=1, space="PSUM") as eps,
            tc.tile_pool(name="pps", bufs=2, space="PSUM") as pps,
            tc.tile_pool(name="aps", bufs=2, space="PSUM") as aps,
        ):
            w = {n: wp.tile([C, C], MDT, tag=n, name=f"w_{n}") for n in w_names}
            sel = wp.tile([4, C], DT, tag="sel")
            cb = wp.tile([C, 1], DT, tag="cb")
            for n in w_names:
                nc.sync.dma_start(w[n][:], w_dram[n][:])
            nc.sync.dma_start(sel[:], sel_dram[:])
            nc.sync.dma_start(cb[:], cb_dram[:])

            for tb in range(T // TB):
                # batched channel-major loads for TB timesteps
                qTb = io.tile([C, TB * N], MDT, tag="qT")
                kTb = io.tile([C, TB * N], MDT, tag="kT")
                vTb = io.tile([C, TB * N], MDT, tag="vT")
                sl = slice(TB * tb, TB * (tb + 1))
                nc.sync.dma_start(qTb[:].rearrange("c (t n) -> c t n", t=TB),
                                  q_in[sl].rearrange("t c n -> c t n"))
                nc.sync.dma_start(kTb[:].rearrange("c (t n) -> c t n", t=TB),
                                  k_in[sl].rearrange("t c n -> c t n"))
                nc.sync.dma_start(vTb[:].rearrange("c (t n) -> c t n", t=TB),
                                  v_in[sl].rearrange("t c n -> c t n"))
                otb = io.tile([C, TB * N], DT, tag="ot")

                for ti in range(TB):
                    qT = qTb[:, ti * N:(ti + 1) * N]
                    kT = kTb[:, ti * N:(ti + 1) * N]
                    vT = vTb[:, ti * N:(ti + 1) * N]

                    # --- projections, padded-head layout --------------------
                    qk_sb = {}
                    for g, (wq_n, wk_n) in (("a", ("wqa", "wka")),
                                            ("b", ("wqb", "wkb"))):
                        pj = pps.tile([C, BANK], DT, tag="proj")
                        nc.tensor.matmul(pj[:, 0:N], w[wq_n][:], qT)
                        nc.tensor.matmul(pj[:, N:2 * N], w[wk_n][:], kT)
                        qk = work.tile([C, 2 * N], MDT, tag=f"qk{g}")
                        nc.vector.tensor_copy(qk[:], pj[:, 0:2 * N])
                        qk_sb[g] = qk

                    # v projection -> node-major v_aug (ones in col 16 of each
                    # 17-wide head block)
                    pv = pps.tile([C, BANK], DT, tag="proj")
                    nc.tensor.matmul(pv[:, 0:C], vT[:, 0:NC0], w["wvt"][:])
                    nc.tensor.matmul(pv[0:NC1, C:2 * C], vT[:, NC0:N], w["wvt"][:])
                    va0 = work.tile([NC0, 17 * H], MDT, tag="va0")
                    va1 = work.tile([NC1, 17 * H], MDT, tag="va1")
                    va0_h = va0[:].rearrange("p (h c) -> p h c", h=H)
                    va1_h = va1[:].rearrange("p (h c) -> p h c", h=H)
                    nc.vector.tensor_copy(
                        va0_h[:, :, 0:D], pv[:, 0:C].rearrange("p (h c) -> p h c", h=H))
                    nc.vector.tensor_copy(
                        va1_h[:, :, 0:D],
                        pv[0:NC1, C:2 * C].rearrange("p (h c) -> p h c", h=H))
                    nc.vector.memset(va0_h[:, :, D:17], 1.0)
                    nc.vector.memset(va1_h[:, :, D:17], 1.0)

                    # --- energy + exp per pad group -------------------------
                    pt_sb = {}
                    for g in ("a", "b"):
                        qk = qk_sb[g]
                        e = eps.tile([C, 4 * BANK], DT, tag="e")
                        for j in range(4):
                            lo = 32 * j
                            q_h = qk[lo:lo + D, 0:N]
                            k_h0 = qk[lo:lo + D, N:N + NC0]
                            k_h1 = qk[lo:lo + D, N + NC0:2 * N]
                            nc.tensor.matmul(e[:, BANK * j:BANK * j + N], k_h0, q_h,
                                             tile_position=(lo, 0))
                            nc.tensor.matmul(e[0:NC1, BANK * j + N:BANK * j + PW],
                                             k_h1, q_h, tile_position=(lo, 0))
                        pt = ptp.tile([C, 4 * PW], MDT, tag="pt")
                        e_view = e[:].rearrange("p (b c) -> p b c", b=4)[:, :, 0:PW]
                        pt_view = pt[:].rearrange("p (b c) -> p b c", b=4)
                        nc.scalar.activation(pt_view, e_view, EXP)
                        pt_sb[g] = pt

                    # --- attn @ v_aug (col-tiled, denom via ones column) ----
                    av = aps.tile([C, BANK], DT, tag="avfin")
                    for gi, g in enumerate(("a", "b")):
                        pt = pt_sb[g]
                        for j in range(4):
                            h = 4 * gi + j
                            co = 32 * j
                            fo = N * gi
                            nc.tensor.matmul(
                                av[co:co + 17, fo:fo + N],
                                va0[:, 17 * h:17 * h + 17],
                                pt[:, PW * j:PW * j + N],
                                start=True, stop=False, tile_position=(0, co))
                            nc.tensor.matmul(
                                av[co:co + 17, fo:fo + N],
                                va1[:, 17 * h:17 * h + 17],
                                pt[0:NC1, PW * j + N:PW * j + PW],
                                start=False, stop=True, tile_position=(0, co))

                    scat = work.tile([C, 2 * N], DT, tag="scat")
                    nc.vector.tensor_copy(scat[:], av[:, 0:2 * N])

                    # --- denominators -> 1/x via exp(-ln(x)) -> expand ------
                    den = work.tile([4, 2 * N], DT, tag="den")
                    for j in range(4):
                        nc.sync.dma_start(den[j:j + 1, :],
                                          scat[32 * j + 16:32 * j + 17, :])
                    lg = work.tile([4, 2 * N], DT, tag="lg")
                    nc.scalar.activation(lg[:], den[:], LOG)
                    rec = work.tile([4, 2 * N], DT, tag="rec")
                    nc.scalar.activation(rec[:], lg[:], EXP, scale=-1.0)
                    rx = aps.tile([C, BANK], DT, tag="avfin")
                    nc.tensor.matmul(rx[:, 0:2 * N], sel[:], rec[:])
                    norm = work.tile([C, 2 * N], MDT, tag="norm")
                    nc.vector.tensor_tensor(norm[:], rx[:, 0:2 * N], scat[:],
                                            mybir.AluOpType.mult)

                    # --- output projection ----------------------------------
                    fin = aps.tile([C, BANK], DT, tag="avfin")
                    nc.tensor.matmul(fin[:, 0:N], w["woa"][:], norm[:, 0:N],
                                     start=True, stop=False)
                    nc.tensor.matmul(fin[:, 0:N], w["wob"][:], norm[:, N:2 * N],
                                     start=False, stop=True)
                    nc.scalar.activation(otb[:, ti * N:(ti + 1) * N], fin[:, 0:N],
                                         IDENT, bias=cb[:, 0:1])
                nc.sync.dma_start(out_dram[sl].rearrange("t c n -> c t n"),
                                  otb[:].rearrange("c (t n) -> c t n", t=TB))

    nc.compile()
    return nc


def _prep_weights(Wv, bv, Wk, bk, Wq, bq, Wo, bo):
    s = F32(1.0 / np.sqrt(C))
    Wq_s = (Wq * s).astype(F32)

    def pad(Wmat, heads):
        out = np.zeros((C, C), F32)
        for j, h in enumerate(heads):
            out[:, 32 * j:32 * j + D] = Wmat[D * h:D * h + D, :].T
        return out

    def wo_pad(heads):
        out = np.zeros((C, C), F32)
        for j, h in enumerate(heads):
            out[32 * j:32 * j + D, :] = Wo[:, D * h:D * h + D].T
        return out

    sel = np.zeros((4, C), F32)
    for j in range(4):
        sel[j, 32 * j:32 * j + 17] = 1.0

    cb = (bo + Wo @ bv).astype(F32).reshape(C, 1)

    if np.any(bq) or np.any(bk):
        raise NotImplementedError(
            "nonzero q/k biases not folded; setup_inputs uses zeros")

    np_mdt = mybir.dt.np(MDT)
    wm = dict(wqa=pad(Wq_s, [0, 1, 2, 3]), wqb=pad(Wq_s, [4, 5, 6, 7]),
              wka=pad(Wk, [0, 1, 2, 3]), wkb=pad(Wk, [4, 5, 6, 7]),
              wvt=np.ascontiguousarray(Wv.T).astype(F32),
              woa=wo_pad([0, 1, 2, 3]), wob=wo_pad([4, 5, 6, 7]))
    wm = {k: v.astype(np_mdt) for k, v in wm.items()}
    wm["sel"] = sel
    wm["cb"] = cb
    return wm


def kernel(values, keys, query, Wv, bv, Wk, bk, Wq, bq, Wo, bo):
    values = np.asarray(values, F32)
    keys = np.asarray(keys, F32)
    query = np.asarray(query, F32)

    if "nc" not in _cache:
        _cache["nc"] = _build_kernel()
    nc = _cache["nc"]

    wmap = _prep_weights(np.asarray(Wv, F32), np.asarray(bv, F32),
                         np.asarray(Wk, F32), np.asarray(bk, F32),
                         np.asarray(Wq, F32), np.asarray(bq, F32),
                         np.asarray(Wo, F32), np.asarray(bo, F32))

    np_mdt = mybir.dt.np(MDT)
    qT = np.ascontiguousarray(query.transpose(0, 1, 3, 2)).astype(np_mdt)
    kT = np.ascontiguousarray(keys.transpose(0, 1, 3, 2)).astype(np_mdt)
    vT = np.ascontiguousarray(values.transpose(0, 1, 3, 2)).astype(np_mdt)

    in_maps = [
        dict(q_in=qT[b], k_in=kT[b], v_in=vT[b], **wmap) for b in range(B)
    ]
    results = run_bass_kernel_spmd(nc, in_maps, list(range(B))).results
    out = np.stack([r["out"] for r in results])             # [B, T, C, N]
    return np.ascontiguousarray(out.transpose(0, 1, 3, 2))  # [B, T, N, C]



# revision 11
# speedup vs baseline: 1.3826x; 1.3826x over previous
"""Spatial multi-head attention kernel for Trainium2 (8 NeuronCores).

Problem: B=8, T=64, N=170 nodes, C=128 channels, H=8 heads, D=16.
Attention over nodes N, independent per (b, t, h).

Strategy:
- Pure data-parallel over B: core b computes batch b fully (no collectives).
- Host transposes inputs to channel-major [B, T, C, N]; output returned
  channel-major [T, C, N] per core and transposed back on host.
- Per (b, t), everything stays channel-major on the device:
  * q/k projections emitted in a "padded head" layout: heads at 32-aligned
    partition bases (4 heads per [128, N] tile) so the K=16 energy matmuls
    are legal row-tiled matmuls running 4 heads concurrently.
  * softmax without max-subtraction (|energy/sqrt(C)| < ~3 in f32): exp on
    ACT directly PSUM -> SBUF, denominators via a ones-column in v_aug.
  * attn@v as outT[d, q] via col-tiled matmuls (M=17).
  * denominator reciprocal as exp(-log(x)) on ACT (DVE reciprocal is
    8 cyc/elem), expanded to [128, 2N] with a constant selector matmul.
  * final projection with head-scattered Wo''; output bias added during the
    PSUM evacuation on ACT.
- Matmul operands in bf16 (PSUM accumulation stays fp32): fp32 matmuls on
  TRN2 run as two HW passes, bf16 runs one and enables fast weight load.
  Set USE_BF16 = False for a full-fp32 build (~2.5x slower, rel err 4e-6
  instead of ~2e-4).
"""
import sys

sys.path.insert(0, "/opt/trn_rl_repo")

import numpy as np

import concourse.bacc as bacc
import concourse.mybir as mybir
import concourse.tile as tile
from concourse.bass_utils import run_bass_kernel_spmd

B, T, N, C = 8, 64, 170, 128
H, D = 8, 16
F32 = np.float32
DT = mybir.dt.float32
EXP = mybir.ActivationFunctionType.Exp
LOG = mybir.ActivationFunctionType.Ln
IDENT = mybir.ActivationFunctionType.Identity

USE_BF16 = True
MDT = mybir.dt.bfloat16 if USE_BF16 else mybir.dt.float32

NC0 = 128          # first k-chunk size
NC1 = N - NC0      # 42
PW = 340           # per-head psum span: chunk0 [0:170], chunk1 [170:340]
BANK = 512         # psum bank free size (f32)
TB = 4             # timesteps per DMA batch

_cache = {}


def _build_kernel():
    nc = bacc.Bacc("TRN2", target_bir_lowering=False, debug=False)

    q_in = nc.dram_tensor("q_in", [T, C, N], MDT, kind="ExternalInput")
    k_in = nc.dram_tensor("k_in", [T, C, N], MDT, kind="ExternalInput")
    v_in = nc.dram_tensor("v_in", [T, C, N], MDT, kind="ExternalInput")
    w_names = ["wqa", "wqb", "wka", "wkb", "wvt", "woa", "wob"]
    w_dram = {n: nc.dram_tensor(n, [C, C], MDT, kind="ExternalInput") for n in w_names}
    sel_dram = nc.dram_tensor("sel", [4, C], MDT, kind="ExternalInput")
    selx_dram = nc.dram_tensor("selx", [C, 4], MDT, kind="ExternalInput")
    cb_dram = nc.dram_tensor("cb", [C, 1], DT, kind="ExternalInput")
    out_dram = nc.dram_tensor("out", [T, C, N], DT, kind="ExternalOutput")

    with tile.TileContext(nc) as tc:
        with (
            tc.tile_pool(name="wp", bufs=1) as wp,
            tc.tile_pool(name="io", bufs=3) as io,
            tc.tile_pool(name="work", bufs=2) as work,
            tc.tile_pool(name="pt", bufs=2) as ptp,
            tc.tile_pool(name="eps", bufs=1, space="PSUM") as eps,
            tc.tile_pool(name="pps", bufs=2, space="PSUM") as pps,
            tc.tile_pool(name="aps", bufs=2, space="PSUM") as aps,
        ):
            w = {n: wp.tile([C, C], MDT, tag=n, name=f"w_{n}") for n in w_names}
            sel = wp.tile([4, C], MDT, tag="sel")
            selx = wp.tile([C, 4], MDT, tag="selx")
            cb = wp.tile([C, 1], DT, tag="cb")
            for n in w_names:
                nc.sync.dma_start(w[n][:], w_dram[n][:])
            nc.sync.dma_start(sel[:], sel_dram[:])
            nc.sync.dma_start(selx[:], selx_dram[:])
            nc.sync.dma_start(cb[:], cb_dram[:])

            for tb in range(T // TB):
                # batched channel-major loads for TB timesteps
                qTb = io.tile([C, TB * N], MDT, tag="qT")
                kTb = io.tile([C, TB * N], MDT, tag="kT")
                vTb = io.tile([C, TB * N], MDT, tag="vT")
                sl = slice(TB * tb, TB * (tb + 1))
                nc.sync.dma_start(qTb[:].rearrange("c (t n) -> c t n", t=TB),
                                  q_in[sl].rearrange("t c n -> c t n"))
                nc.sync.dma_start(kTb[:].rearrange("c (t n) -> c t n", t=TB),
                                  k_in[sl].rearrange("t c n -> c t n"))
                nc.sync.dma_start(vTb[:].rearrange("c (t n) -> c t n", t=TB),
                                  v_in[sl].rearrange("t c n -> c t n"))
                otb = io.tile([C, TB * N], DT, tag="ot")

                for ti in range(TB):
                    qT = qTb[:, ti * N:(ti + 1) * N]
                    kT = kTb[:, ti * N:(ti + 1) * N]
                    vT = vTb[:, ti * N:(ti + 1) * N]

                    # --- projections, padded-head layout --------------------
                    qk_sb = {}
                    for g, (wq_n, wk_n) in (("a", ("wqa", "wka")),
                                            ("b", ("wqb", "wkb"))):
                        pj = pps.tile([C, BANK], DT, tag="proj")
                        nc.tensor.matmul(pj[:, 0:N], w[wq_n][:], qT)
                        nc.tensor.matmul(pj[:, N:2 * N], w[wk_n][:], kT)
                        qk = work.tile([C, 2 * N], MDT, tag=f"qk{g}")
                        nc.vector.tensor_copy(qk[:], pj[:, 0:2 * N])
                        qk_sb[g] = qk

                    # v projection -> node-major v_aug (ones in col 16 of each
                    # 17-wide head block)
                    pv = pps.tile([C, BANK], DT, tag="proj")
                    nc.tensor.matmul(pv[:, 0:C], vT[:, 0:NC0], w["wvt"][:])
                    nc.tensor.matmul(pv[0:NC1, C:2 * C], vT[:, NC0:N], w["wvt"][:])
                    va0 = work.tile([NC0, 17 * H], MDT, tag="va0")
                    va1 = work.tile([NC1, 17 * H], MDT, tag="va1")
                    va0_h = va0[:].rearrange("p (h c) -> p h c", h=H)
                    va1_h = va1[:].rearrange("p (h c) -> p h c", h=H)
                    nc.vector.tensor_copy(
                        va0_h[:, :, 0:D], pv[:, 0:C].rearrange("p (h c) -> p h c", h=H))
                    nc.vector.tensor_copy(
                        va1_h[:, :, 0:D],
                        pv[0:NC1, C:2 * C].rearrange("p (h c) -> p h c", h=H))
                    nc.vector.memset(va0_h[:, :, D:17], 1.0)
                    nc.vector.memset(va1_h[:, :, D:17], 1.0)

                    # --- energy + exp per pad group -------------------------
                    pt_sb = {}
                    for g in ("a", "b"):
                        qk = qk_sb[g]
                        e = eps.tile([C, 4 * BANK], DT, tag="e")
                        for j in range(4):
                            lo = 32 * j
                            q_h = qk[lo:lo + D, 0:N]
                            k_h0 = qk[lo:lo + D, N:N + NC0]
                            k_h1 = qk[lo:lo + D, N + NC0:2 * N]
                            nc.tensor.matmul(e[:, BANK * j:BANK * j + N], k_h0, q_h,
                                             tile_position=(lo, 0))
                            nc.tensor.matmul(e[0:NC1, BANK * j + N:BANK * j + PW],
                                             k_h1, q_h, tile_position=(lo, 0))
                        pt = ptp.tile([C, 4 * PW], MDT, tag="pt")
                        e_view = e[:].rearrange("p (b c) -> p b c", b=4)[:, :, 0:PW]
                        pt_view = pt[:].rearrange("p (b c) -> p b c", b=4)
                        nc.scalar.activation(pt_view, e_view, EXP)
                        pt_sb[g] = pt

                    # --- attn @ v_aug (col-tiled, denom via ones column) ----
                    av = aps.tile([C, BANK], DT, tag="avfin")
                    for gi, g in enumerate(("a", "b")):
                        pt = pt_sb[g]
                        for j in range(4):
                            h = 4 * gi + j
                            co = 32 * j
                            fo = N * gi
                            nc.tensor.matmul(
                                av[co:co + 17, fo:fo + N],
                                va0[:, 17 * h:17 * h + 17],
                                pt[:, PW * j:PW * j + N],
                                start=True, stop=False, tile_position=(0, co))
                            nc.tensor.matmul(
                                av[co:co + 17, fo:fo + N],
                                va1[:, 17 * h:17 * h + 17],
                                pt[0:NC1, PW * j + N:PW * j + PW],
                                start=False, stop=True, tile_position=(0, co))

                    scat = work.tile([C, 2 * N], MDT, tag="scat")
                    nc.vector.tensor_copy(scat[:], av[:, 0:2 * N])

                    # --- denominators: extract rows via matmul, 1/x on DVE --
                    den_ps = aps.tile([C, BANK], DT, tag="avfin")
                    nc.tensor.matmul(den_ps[0:4, 0:2 * N], selx[:], scat[:])
                    rec = work.tile([4, 2 * N], DT, tag="rec")
                    nc.vector.reciprocal_approx_fast(rec[:], den_ps[0:4, 0:2 * N])
                    rec_bf = work.tile([4, 2 * N], MDT, tag="recb")
                    nc.vector.tensor_copy(rec_bf[:], rec[:])
                    rx = aps.tile([C, BANK], DT, tag="avfin")
                    nc.tensor.matmul(rx[:, 0:2 * N], sel[:], rec_bf[:])
                    norm = work.tile([C, 2 * N], MDT, tag="norm")
                    nc.vector.tensor_tensor(norm[:], rx[:, 0:2 * N], scat[:],
                                            mybir.AluOpType.mult)

                    # --- output projection ----------------------------------
                    fin = aps.tile([C, BANK], DT, tag="avfin")
                    nc.tensor.matmul(fin[:, 0:N], w["woa"][:], norm[:, 0:N],
                                     start=True, stop=False)
                    nc.tensor.matmul(fin[:, 0:N], w["wob"][:], norm[:, N:2 * N],
                                     start=False, stop=True)
                    nc.vector.tensor_scalar_add(otb[:, ti * N:(ti + 1) * N],
                                                fin[:, 0:N], cb[:, 0:1])
                nc.sync.dma_start(out_dram[sl].rearrange("t c n -> c t n"),
                                  otb[:].rearrange("c (t n) -> c t n", t=TB))

    nc.compile()
    return nc


def _prep_weights(Wv, bv, Wk, bk, Wq, bq, Wo, bo):
    s = F32(1.0 / np.sqrt(C))
    Wq_s = (Wq * s).astype(F32)

    def pad(Wmat, heads):
        out = np.zeros((C, C), F32)
        for j, h in enumerate(heads):
            out[:, 32 * j:32 * j + D] = Wmat[D * h:D * h + D, :].T
        return out

    def wo_pad(heads):
        out = np.zeros((C, C), F32)
        for j, h in enumerate(heads):
            out[32 * j:32 * j + D, :] = Wo[:, D * h:D * h + D].T
        return out

    np_mdt0 = mybir.dt.np(MDT)
    sel = np.zeros((4, C), np_mdt0)
    selx = np.zeros((C, 4), np_mdt0)
    for j in range(4):
        sel[j, 32 * j:32 * j + 17] = 1.0
        selx[32 * j + 16, j] = 1.0

    cb = (bo + Wo @ bv).astype(F32).reshape(C, 1)

    if np.any(bq) or np.any(bk):
        raise NotImplementedError(
            "nonzero q/k biases not folded; setup_inputs uses zeros")

    np_mdt = mybir.dt.np(MDT)
    wm = dict(wqa=pad(Wq_s, [0, 1, 2, 3]), wqb=pad(Wq_s, [4, 5, 6, 7]),
              wka=pad(Wk, [0, 1, 2, 3]), wkb=pad(Wk, [4, 5, 6, 7]),
              wvt=np.ascontiguousarray(Wv.T).astype(F32),
              woa=wo_pad([0, 1, 2, 3]), wob=wo_pad([4, 5, 6, 7]))
    wm = {k: v.astype(np_mdt) for k, v in wm.items()}
    wm["sel"] = sel
    wm["selx"] = selx
    wm["cb"] = cb
    return wm


def kernel(values, keys, query, Wv, bv, Wk, bk, Wq, bq, Wo, bo):
    values = np.asarray(values, F32)
    keys = np.asarray(keys, F32)
    query = np.asarray(query, F32)

    if "nc" not in _cache:
        _cache["nc"] = _build_kernel()
    nc = _cache["nc"]

    wmap = _prep_weights(np.asarray(Wv, F32), np.asarray(bv, F32),
                         np.asarray(Wk, F32), np.asarray(bk, F32),
                         np.asarray(Wq, F32), np.asarray(bq, F32),
                         np.asarray(Wo, F32), np.asarray(bo, F32))

    np_mdt = mybir.dt.np(MDT)
    qT = np.ascontiguousarray(query.transpose(0, 1, 3, 2)).astype(np_mdt)
    kT = np.ascontiguousarray(keys.transpose(0, 1, 3, 2)).astype(np_mdt)
    vT = np.ascontiguousarray(values.transpose(0, 1, 3, 2)).astype(np_mdt)

    in_maps = [
        dict(q_in=qT[b], k_in=kT[b], v_in=vT[b], **wmap) for b in range(B)
    ]
    results = run_bass_kernel_spmd(nc, in_maps, list(range(B))).results
    out = np.stack([r["out"] for r in results])             # [B, T, C, N]
    return np.ascontiguousarray(out.transpose(0, 1, 3, 2))  # [B, T, N, C]

